# revision 1
# baseline (speedup 1.0000x reference)
"""Pairwise squared Euclidean distance kernel for Trainium2 (8 NeuronCores).

dist[b, c] = ||feat[b] - centers[c]||^2 = x2[b] + c2[c] - 2 * feat @ centers.T

Sharding: data-parallel along B. Each of the 8 cores gets feat rows
[i*2048, (i+1)*2048), full centers replicated, and produces its
[2048, 4096] block of the output.

Per-core kernel (roofline: 1024 f32r matmuls x ~227 ns ~= 232 us):
  - GEMM in float32r (TF32): the PE rounds fp32 operands on read at full
    1 cyc/row rate (vs 4 cyc/row for fp32); ~2e-5 scale-relative error.
  - featT shard (8 MB) becomes fully SBUF-resident during pass 0, in 8
    per-super-tile tiles; later passes reuse it (no re-streaming).
  - centersT is processed in 4 n-blocks of 1024 columns through a
    2-slot ring; block b+1 streams in while block b computes. Only
    ~5 MB of DMA (first featT block + ct block 0, k-interleaved) gates
    the first matmul.
  - x2 / c2 row norms are host-side input prep (0.02% of the FLOPs);
    c2 arrives pre-replicated [128, C].
  - Epilogue per [128, 512] tile: ACT Identity(scale=-2, bias=x2[m])
    PSUM->SBUF (frees the bank), DVE += c2, DMA out.
  - DMAs execute in emission order; all loads are emission-placed so
    data arrives just ahead of use (head k-interleave, ft_{sm+1} and
    ct_{b+1} prefetched inside the loops).
"""
import sys

if "/opt/trn_rl_repo" not in sys.path:
    sys.path.insert(0, "/opt/trn_rl_repo")

import numpy as np

import concourse.bass as bass
import concourse.mybir as mybir
import concourse.tile as tile
from concourse import bacc
from concourse.bass_utils import run_bass_kernel_spmd


def _install_ntff_hook() -> bool:
    """The agent image's `antenv` lacks `axon_hooks`, so bass_utils' NTFF
    trace path crashes on import. Provide the module and register the
    ctypes-based hook against the axon PJRT .so (same recipe as
    trn_agent_boot.trn_boot)."""
    try:
        import types
        import antenv
        if "antenv.axon_hooks" not in sys.modules:
            mod = types.ModuleType("antenv.axon_hooks")
            mod._hook = None
            def set_axon_ntff_profile_hook(h):
                mod._hook = h
            def get_axon_ntff_profile_hook():
                return mod._hook
            mod.set_axon_ntff_profile_hook = set_axon_ntff_profile_hook
            mod.get_axon_ntff_profile_hook = get_axon_ntff_profile_hook
            sys.modules["antenv.axon_hooks"] = mod
            antenv.axon_hooks = mod
        mod = sys.modules["antenv.axon_hooks"]
        if mod._hook is None:
            from trn_agent_boot.trn_boot import _ntff_profile_via_ctypes
            hook = _ntff_profile_via_ctypes("/opt/axon/libaxon_pjrt.so")
            if hook is None:
                return False
            mod.set_axon_ntff_profile_hook(hook)
        return True
    except Exception as e:  # profiling is best-effort
        print(f"NTFF hook install failed: {e}", file=sys.stderr)
        return False


B, C, D = 16384, 4096, 1024
N_CORES = 8
BS = B // N_CORES            # 2048 feat rows per core
KT = D // 128                # 8 k-tiles
MT = BS // 128               # 16 m-tiles per core
NB = 4                       # n-blocks (passes over n)
CB = C // NB                 # 1024 n-columns per block
NT = CB // 512               # 2 n-tiles of 512 per block
M_SUPER = 2                  # m-tiles per featT tile (256 cols)
SM = MT // M_SUPER           # 8 featT super-tiles

F32 = mybir.dt.float32
F32R = mybir.dt.float32r

LAST = {"exec_time_ns": None, "mean_exec_time_ns": None}


def _build():
    nc = bacc.Bacc("TRN2", target_bir_lowering=False, debug=False,
                   num_devices=N_CORES)
    d_featT = nc.dram_tensor("featT", [D, BS], F32, kind="ExternalInput").ap()
    d_centersT = nc.dram_tensor("centersT", [D, C], F32, kind="ExternalInput").ap()
    d_c2b = nc.dram_tensor("c2b", [128, C], F32, kind="ExternalInput").ap()
    d_x2 = nc.dram_tensor("x2", [128, MT], F32, kind="ExternalInput").ap()
    d_dist = nc.dram_tensor("dist", [BS, C], F32, kind="ExternalOutput").ap()

    featT_pkm = d_featT.rearrange("(kt p) m -> p kt m", p=128)
    centersT_pkn = d_centersT.rearrange("(kt p) n -> p kt n", p=128)

    with tile.TileContext(nc) as tc:
        with tc.tile_pool(name="cpool", bufs=1) as cpool, \
             tc.tile_pool(name="ctp", bufs=2) as ctp, \
             tc.tile_pool(name="opool", bufs=8) as opool, \
             tc.tile_pool(name="psp", bufs=3, space="PSUM") as psp:
            # persistent featT tiles, one per super-tile of 256 feat rows
            fts = [cpool.tile([128, KT, 128 * M_SUPER], F32R, name=f"ft{s}")
                   for s in range(SM)]
            x2all = cpool.tile([128, MT], F32, name="x2all")
            c2b = cpool.tile([128, C], F32, name="c2b")

            # Sync-engine DMA dispatch costs ~650 ns per dma_start, so
            # transfers are batched into few instructions.
            def load_ft(sm):
                nc.sync.dma_start(
                    fts[sm][:],
                    featT_pkm[:, :, bass.ts(sm, 128 * M_SUPER)].bitcast(F32R))

            # head: ft super-tile 0 first, then ct block 0's k-tiles so
            # m-tile 0's k-loop is paced by arrivals; ft1/ft2 right after
            # so m-tiles 2..5 never wait; c2b (first needed by m-tile 0's
            # epilogue, which has osb-pool slack) goes last
            ct_cur = ctp.tile([128, KT, CB], F32R, name="ctblk")
            # k0 slices of ft0/ct0 first: the first matmul needs only these
            nc.sync.dma_start(fts[0][:, 0, :],
                              featT_pkm[:, 0, 0:128 * M_SUPER].bitcast(F32R))
            nc.sync.dma_start(ct_cur[:, 0, :],
                              centersT_pkn[:, 0, 0:CB].bitcast(F32R))
            nc.sync.dma_start(fts[0][:, 1:KT, :],
                              featT_pkm[:, 1:KT, 0:128 * M_SUPER].bitcast(F32R))
            for k in range(1, KT):
                nc.sync.dma_start(
                    ct_cur[:, k, :], centersT_pkn[:, k, 0:CB].bitcast(F32R))
            load_ft(1)
            nc.sync.dma_start(x2all[:], d_x2)
            load_ft(2)
            nc.sync.dma_start(c2b[:], d_c2b)

            # HAM warm-up: ~9us of dummy matmuls on a memset tile while the
            # head DMAs are in flight, so real matmuls start at 2.4 GHz
            wsrc = cpool.tile([128, 512], F32, name="wsrc")
            nc.vector.memset(wsrc[:], 0.5)
            wsrc_r = cpool.tile([128, 512], F32R, name="wsrc_r")
            nc.vector.tensor_copy(wsrc_r[:], wsrc[:])
            pd = psp.tile([128, 512], F32, name="pd", bufs=1)
            for w in range(20):
                nc.tensor.matmul(pd[:], wsrc_r[:, 0:128], wsrc_r[:],
                                 start=True, stop=True)

            for pb in range(NB):
                ct_next = None
                if pb + 1 < NB:
                    ct_next = ctp.tile([128, KT, CB], F32R, name="ctblk")
                for sm in range(SM):
                    if pb == 0 and sm + 3 < SM:
                        load_ft(sm + 3)     # prefetch featT two super-tiles out
                    if ct_next is not None and sm in (4, 5):
                        # prefetch next ct block mid-pass in two half-transfers
                        kh = slice(0, 4) if sm == 4 else slice(4, 8)
                        nc.sync.dma_start(
                            ct_next[:, kh, :],
                            centersT_pkn[:, kh, bass.ts(pb + 1, CB)].bitcast(F32R))
                    for mi in range(M_SUPER):
                        mt = sm * M_SUPER + mi
                        pss = [psp.tile([128, 512], F32, name=f"ps{n}")
                               for n in range(NT)]
                        for k in range(KT):
                            lhs = fts[sm][:, k, bass.ts(mi, 128)]
                            for n in range(NT):
                                nc.tensor.matmul(pss[n][:], lhs,
                                                 ct_cur[:, k, bass.ts(n, 512)],
                                                 start=(k == 0), stop=(k == KT - 1))
                            if pb == 0 and mt == 0 and k < KT - 1:
                                # m-tile 0's k-loop is paced by ct DMA
                                # arrivals; fill the ~1us gaps with dummy
                                # matmuls so HAM never re-throttles
                                for w in range(3):
                                    nc.tensor.matmul(pd[:], wsrc_r[:, 0:128],
                                                     wsrc_r[:],
                                                     start=True, stop=True)
                        osb = opool.tile([128, CB], F32, name="osb")
                        for n in range(NT):
                            gn = pb * CB + n * 512   # global n offset
                            nc.scalar.activation(
                                osb[:, bass.ts(n, 512)], pss[n][:],
                                mybir.ActivationFunctionType.Identity,
                                bias=x2all[:, mt:mt + 1], scale=-2.0)
                            nc.vector.tensor_add(osb[:, bass.ts(n, 512)],
                                                 osb[:, bass.ts(n, 512)],
                                                 c2b[:, gn:gn + 512])
                        nc.sync.dma_start(
                            d_dist[bass.ts(mt, 128), bass.ts(pb, CB)], osb[:])
                ct_cur = ct_next

            # sink read so the warm-up/dummy matmuls aren't dead-code
            wsink = cpool.tile([128, 1], F32, name="wsink")
            nc.scalar.copy(wsink[:], pd[:, 0:1])

    nc.compile()
    return nc


def kernel(feat: np.ndarray, centers: np.ndarray, *, trace: bool = False) -> np.ndarray:
    feat = np.ascontiguousarray(np.asarray(feat, dtype=np.float32))
    centers = np.ascontiguousarray(np.asarray(centers, dtype=np.float32))
    assert feat.shape == (B, D) and centers.shape == (C, D)

    featT = np.ascontiguousarray(feat.T)          # [D, B]
    centersT = np.ascontiguousarray(centers.T)    # [D, C]
    c2 = (centers.astype(np.float64) ** 2).sum(axis=1).astype(np.float32)
    c2b = np.ascontiguousarray(np.broadcast_to(c2[None, :], (128, C)))
    x2 = (feat.astype(np.float64) ** 2).sum(axis=1).astype(np.float32)

    in_maps = []
    for i in range(N_CORES):
        sl = slice(i * BS, (i + 1) * BS)
        in_maps.append({
            "featT": np.ascontiguousarray(featT[:, sl]),
            "centersT": centersT,
            "c2b": c2b,
            # x2 shard laid out [128, MT]: column mt holds rows of m-tile mt
            "x2": np.ascontiguousarray(x2[sl].reshape(MT, 128).T),
        })

    if trace:
        trace = _install_ntff_hook()

    nc = _build()
    res = None
    for attempt in range(3):
        try:
            res = run_bass_kernel_spmd(nc, in_maps,
                                       core_ids=list(range(N_CORES)),
                                       trace=trace)
            break
        except Exception as e:
            # transient NRT/axon device faults recover on retry
            if attempt == 2:
                raise
            print(f"kernel run attempt {attempt} failed ({e}); retrying",
                  file=sys.stderr)
    LAST["exec_time_ns"] = res.exec_time_ns
    LAST["mean_exec_time_ns"] = res.mean_exec_time_ns

    out = np.empty((B, C), dtype=np.float32)
    for i in range(N_CORES):
        out[i * BS:(i + 1) * BS] = res.results[i]["dist"]
    return out


if __name__ == "__main__":
    rng = np.random.default_rng(0)
    f = rng.standard_normal((B, D), dtype=np.float32)
    c = rng.standard_normal((C, D), dtype=np.float32)
    d = kernel(f, c, trace=True)
    print("exec_time_ns:", LAST["exec_time_ns"])



# revision 3
# speedup vs baseline: 2.0462x; 2.0462x over previous
"""Pairwise squared Euclidean distance kernel for Trainium2 (8 NeuronCores).

dist[b, c] = ||feat[b] - centers[c]||^2 = x2[b] + c2[c] - 2 * feat @ centers.T

Sharding: data-parallel along B. Each of the 8 cores gets feat rows
[i*2048, (i+1)*2048), full centers replicated, and produces its
[2048, 4096] block of the output.

Per-core kernel (FP8 DoubleRow; roofline ~109 us streaming):
  - GEMM in fp8 e4m3 with perf_mode=DoubleRow: the PE packs 2 fp8
    weights per cell (virtual 256-deep contraction), 2 MACs/cell/cyc.
    512 matmuls x [K=256, M=128, N=512] at ~0.5 cyc/moving-row.
    Empirical accuracy vs fp32 reference: ~5.4e-3 scale-relative
    (inputs are N(0,1); e4m3 has 3 mantissa bits; accumulate is fp32).
  - x2 / c2 row norms are exact (fp64 host prep); dist assembled as
    -2*xc + x2 (ACT, bias per partition) + c2 (DVE, fp16 2x mode).
  - Output stored fp16 (abs err ~0.5 vs values ~2e3), upcast on host.
    Halves the output DMA (16 MB/core).
  - All inputs SBUF-resident in fp8: featT 2 MB + centersT 4 MB +
    c2b fp16 1 MB; host pre-arranges DRAM so every DMA lands as
    contiguous >=1KB per-partition runs matching SBUF tile layout.
  - DMAs execute in emission order; head k-paced so m-tile 0's k-loop
    starts ~2 us in; HAM warm-up dummies cover the cold 1.2 GHz window.
"""
import sys

if "/opt/trn_rl_repo" not in sys.path:
    sys.path.insert(0, "/opt/trn_rl_repo")

import ml_dtypes
import numpy as np

import concourse.bass as bass
import concourse.mybir as mybir
import concourse.tile as tile
from concourse import bacc
from concourse.bass_utils import run_bass_kernel_spmd


def _install_ntff_hook() -> bool:
    """The agent image's `antenv` lacks `axon_hooks`, so bass_utils' NTFF
    trace path crashes on import. Provide the module and register the
    ctypes-based hook against the axon PJRT .so (same recipe as
    trn_agent_boot.trn_boot)."""
    try:
        import types
        import antenv
        if "antenv.axon_hooks" not in sys.modules:
            mod = types.ModuleType("antenv.axon_hooks")
            mod._hook = None
            def set_axon_ntff_profile_hook(h):
                mod._hook = h
            def get_axon_ntff_profile_hook():
                return mod._hook
            mod.set_axon_ntff_profile_hook = set_axon_ntff_profile_hook
            mod.get_axon_ntff_profile_hook = get_axon_ntff_profile_hook
            sys.modules["antenv.axon_hooks"] = mod
            antenv.axon_hooks = mod
        mod = sys.modules["antenv.axon_hooks"]
        if mod._hook is None:
            from trn_agent_boot.trn_boot import _ntff_profile_via_ctypes
            hook = _ntff_profile_via_ctypes("/opt/axon/libaxon_pjrt.so")
            if hook is None:
                return False
            mod.set_axon_ntff_profile_hook(hook)
        return True
    except Exception as e:  # profiling is best-effort
        print(f"NTFF hook install failed: {e}", file=sys.stderr)
        return False


B, C, D = 16384, 4096, 1024
N_CORES = 8
BS = B // N_CORES            # 2048 feat rows per core
KT = D // 128                # 8 k-tiles of 128
KS = KT // 2                 # 4 DoubleRow k-super-tiles of 256
MT = BS // 128               # 16 m-tiles per core
NB = 4                       # n-blocks (passes over n)
CB = C // NB                 # 1024 n-columns per block
NT = CB // 512               # 2 n-tiles of 512 per block
M_SUPER = 2                  # m-tiles per featT tile (256 cols)
SM = MT // M_SUPER           # 8 featT super-tiles

F32 = mybir.dt.float32
F16 = mybir.dt.float16
F32R = mybir.dt.float32r
FP8 = mybir.dt.float8e4
DR = mybir.MatmulPerfMode.DoubleRow

LAST = {"exec_time_ns": None, "mean_exec_time_ns": None}


def _build():
    nc = bacc.Bacc("TRN2", target_bir_lowering=False, debug=False,
                   num_devices=N_CORES)
    # DRAM layouts match the SBUF tile layouts exactly (host pre-arranged)
    d_featT = nc.dram_tensor("featT", [128, SM * KT * 256], FP8,
                             kind="ExternalInput").ap()
    d_centersT = nc.dram_tensor("centersT", [128, NB * KT * CB], FP8,
                                kind="ExternalInput").ap()
    d_c2b = nc.dram_tensor("c2b", [128, C], F16, kind="ExternalInput").ap()
    d_x2 = nc.dram_tensor("x2", [128, MT], F32, kind="ExternalInput").ap()
    d_dist = nc.dram_tensor("dist", [BS, C], F16, kind="ExternalOutput").ap()

    featT_src = d_featT.rearrange("p (sm kt m) -> p sm kt m", sm=SM, kt=KT)
    ct_src = d_centersT.rearrange("p (nb kt n) -> p nb kt n", nb=NB, kt=KT)

    with tile.TileContext(nc) as tc:
        with tc.tile_pool(name="cpool", bufs=1) as cpool, \
             tc.tile_pool(name="opool", bufs=8) as opool, \
             tc.tile_pool(name="psp", bufs=3, space="PSUM") as psp:
            # fully SBUF-resident fp8 operands
            fts = [cpool.tile([128, KT, 256], FP8, name=f"ft{s}")
                   for s in range(SM)]
            cts = [cpool.tile([128, KT, CB], FP8, name=f"ct{b}")
                   for b in range(NB)]
            x2all = cpool.tile([128, MT], F32, name="x2all")
            c2b = cpool.tile([128, C], F16, name="c2b")

            # head: first matmul needs ft0 kt0-1 + ct0 kt0-1 only; pace the
            # rest of block 0's k-tiles, then everything else.
            nc.sync.dma_start(fts[0][:, 0:2, :], featT_src[:, 0, 0:2, :])
            nc.sync.dma_start(cts[0][:, 0:2, :], ct_src[:, 0, 0:2, :])
            nc.sync.dma_start(fts[0][:, 2:KT, :], featT_src[:, 0, 2:KT, :])
            nc.sync.dma_start(cts[0][:, 2:4, :], ct_src[:, 0, 2:4, :])
            nc.sync.dma_start(cts[0][:, 4:6, :], ct_src[:, 0, 4:6, :])
            nc.sync.dma_start(cts[0][:, 6:8, :], ct_src[:, 0, 6:8, :])
            nc.sync.dma_start(fts[1][:], featT_src[:, 1, :, :])
            nc.sync.dma_start(x2all[:], d_x2)
            nc.sync.dma_start(fts[2][:], featT_src[:, 2, :, :])
            nc.sync.dma_start(c2b[:], d_c2b)
            for s in range(3, SM):
                nc.sync.dma_start(fts[s][:], featT_src[:, s, :, :])
            for b in range(1, NB):
                nc.sync.dma_start(cts[b][:], ct_src[:, b, :, :])

            # HAM warm-up: dummy matmuls on a memset tile while the head
            # DMAs are in flight, so real matmuls run at 2.4 GHz
            wsrc = cpool.tile([128, 512], F32, name="wsrc")
            nc.vector.memset(wsrc[:], 0.5)
            wsrc_r = cpool.tile([128, 512], F32R, name="wsrc_r")
            nc.vector.tensor_copy(wsrc_r[:], wsrc[:])
            pd = psp.tile([128, 512], F32, name="pd", bufs=1)
            for w in range(12):
                nc.tensor.matmul(pd[:], wsrc_r[:, 0:128], wsrc_r[:],
                                 start=True, stop=True)

            for pb in range(NB):
                ct = cts[pb]
                for sm in range(SM):
                    for mi in range(M_SUPER):
                        mt = sm * M_SUPER + mi
                        pss = [psp.tile([128, 512], F32, name=f"ps{n}")
                               for n in range(NT)]
                        for kk in range(KS):
                            lhs = fts[sm][:, 2 * kk:2 * kk + 2,
                                          bass.ts(mi, 128)]
                            for n in range(NT):
                                nc.tensor.matmul(
                                    pss[n][:], lhs,
                                    ct[:, 2 * kk:2 * kk + 2, bass.ts(n, 512)],
                                    start=(kk == 0), stop=(kk == KS - 1),
                                    perf_mode=DR)
                            if pb == 0 and mt == 0 and kk < KS - 1:
                                # m-tile 0's k-loop is paced by ct DMA
                                # arrivals; fill gaps with dummy matmuls so
                                # HAM never re-throttles
                                for w in range(2):
                                    nc.tensor.matmul(pd[:], wsrc_r[:, 0:128],
                                                     wsrc_r[:],
                                                     start=True, stop=True)
                        osb = opool.tile([128, CB], F16, name="osb")
                        for n in range(NT):
                            gn = pb * CB + n * 512   # global n offset
                            nc.scalar.activation(
                                osb[:, bass.ts(n, 512)], pss[n][:],
                                mybir.ActivationFunctionType.Identity,
                                bias=x2all[:, mt:mt + 1], scale=-2.0)
                            nc.vector.tensor_add(osb[:, bass.ts(n, 512)],
                                                 osb[:, bass.ts(n, 512)],
                                                 c2b[:, gn:gn + 512])
                        nc.sync.dma_start(
                            d_dist[bass.ts(mt, 128), bass.ts(pb, CB)], osb[:])

            # sink read so the warm-up/dummy matmuls aren't dead-code
            wsink = cpool.tile([128, 1], F32, name="wsink")
            nc.scalar.copy(wsink[:], pd[:, 0:1])

    nc.compile()
    return nc


def kernel(feat: np.ndarray, centers: np.ndarray, *, trace: bool = False) -> np.ndarray:
    feat = np.ascontiguousarray(np.asarray(feat, dtype=np.float32))
    centers = np.ascontiguousarray(np.asarray(centers, dtype=np.float32))
    assert feat.shape == (B, D) and centers.shape == (C, D)

    FP8NP = ml_dtypes.float8_e4m3          # TRN e4m3 variant (max +-240)
    feat8 = feat.astype(FP8NP)             # RNE; |x| <= ~6 so no overflow
    centers8 = centers.astype(FP8NP)

    # exact row norms (the fp8 rounding only affects the cross term)
    c2 = (centers.astype(np.float64) ** 2).sum(axis=1).astype(np.float32)
    c2b = np.ascontiguousarray(
        np.broadcast_to(c2[None, :], (128, C))).astype(np.float16)
    x2 = (feat.astype(np.float64) ** 2).sum(axis=1).astype(np.float32)

    # centersT arranged [p][nb][kt][n'] so block DMAs are contiguous
    ctT = np.ascontiguousarray(centers8.T)                   # [D, C]
    ct_arr = np.ascontiguousarray(
        ctT.reshape(KT, 128, NB, CB).transpose(1, 2, 0, 3)).reshape(128, -1)

    in_maps = []
    for i in range(N_CORES):
        sl = slice(i * BS, (i + 1) * BS)
        ftT = np.ascontiguousarray(feat8[sl].T)              # [D, BS]
        ft_arr = np.ascontiguousarray(
            ftT.reshape(KT, 128, SM, 256).transpose(1, 2, 0, 3)
        ).reshape(128, -1)
        in_maps.append({
            "featT": ft_arr,
            "centersT": ct_arr,
            "c2b": c2b,
            # x2 shard laid out [128, MT]: column mt holds rows of m-tile mt
            "x2": np.ascontiguousarray(x2[sl].reshape(MT, 128).T),
        })

    if trace:
        trace = _install_ntff_hook()

    nc = _build()
    res = None
    for attempt in range(3):
        try:
            res = run_bass_kernel_spmd(nc, in_maps,
                                       core_ids=list(range(N_CORES)),
                                       trace=trace)
            break
        except Exception as e:
            # transient NRT/axon device faults recover on retry
            if attempt == 2:
                raise
            print(f"kernel run attempt {attempt} failed ({e}); retrying",
                  file=sys.stderr)
    LAST["exec_time_ns"] = res.exec_time_ns
    LAST["mean_exec_time_ns"] = res.mean_exec_time_ns

    out = np.empty((B, C), dtype=np.float32)
    for i in range(N_CORES):
        out[i * BS:(i + 1) * BS] = res.results[i]["dist"].astype(np.float32)
    return out


if __name__ == "__main__":
    rng = np.random.default_rng(0)
    f = rng.standard_normal((B, D), dtype=np.float32)
    c = rng.standard_normal((C, D), dtype=np.float32)
    d = kernel(f, c, trace=True)
    print("exec_time_ns:", LAST["exec_time_ns"])


# revision 6
# speedup vs baseline: 2.0741x; 1.0137x over previous
"""Pairwise squared Euclidean distance kernel for Trainium2 (8 NeuronCores).

dist[b, c] = ||feat[b] - centers[c]||^2 = x2[b] + c2[c] - 2 * feat @ centers.T

Sharding: data-parallel along B. Each of the 8 cores gets feat rows
[i*2048, (i+1)*2048), full centers replicated, and produces its
[2048, 4096] block of the output.

Per-core kernel (FP8 DoubleRow; roofline ~109 us streaming):
  - GEMM in fp8 e4m3 with perf_mode=DoubleRow: the PE packs 2 fp8
    weights per cell (virtual 256-deep contraction), 2 MACs/cell/cyc.
    512 matmuls x [K=256, M=128, N=512] at ~0.5 cyc/moving-row.
    Empirical accuracy vs fp32 reference: ~5.4e-3 scale-relative
    (inputs are N(0,1); e4m3 has 3 mantissa bits; accumulate is fp32).
  - x2 / c2 row norms are exact (fp64 host prep); dist assembled as
    -2*xc + x2 (ACT, bias per partition) + c2 (DVE, fp16 2x mode).
  - Output stored fp16 (abs err ~0.5 vs values ~2e3), upcast on host.
    Halves the output DMA (16 MB/core).
  - All inputs SBUF-resident in fp8: featT 2 MB + centersT 4 MB +
    c2b fp16 1 MB; host pre-arranges DRAM so every DMA lands as
    contiguous >=1KB per-partition runs matching SBUF tile layout.
  - DMAs execute in emission order; head k-paced so m-tile 0's k-loop
    starts ~2 us in; HAM warm-up dummies cover the cold 1.2 GHz window.
"""
import sys

if "/opt/trn_rl_repo" not in sys.path:
    sys.path.insert(0, "/opt/trn_rl_repo")

import ml_dtypes
import numpy as np

import concourse.bass as bass
import concourse.mybir as mybir
import concourse.tile as tile
from concourse import bacc
from concourse.bass_utils import run_bass_kernel_spmd


def _install_ntff_hook() -> bool:
    """The agent image's `antenv` lacks `axon_hooks`, so bass_utils' NTFF
    trace path crashes on import. Provide the module and register the
    ctypes-based hook against the axon PJRT .so (same recipe as
    trn_agent_boot.trn_boot)."""
    try:
        import types
        import antenv
        if "antenv.axon_hooks" not in sys.modules:
            mod = types.ModuleType("antenv.axon_hooks")
            mod._hook = None
            def set_axon_ntff_profile_hook(h):
                mod._hook = h
            def get_axon_ntff_profile_hook():
                return mod._hook
            mod.set_axon_ntff_profile_hook = set_axon_ntff_profile_hook
            mod.get_axon_ntff_profile_hook = get_axon_ntff_profile_hook
            sys.modules["antenv.axon_hooks"] = mod
            antenv.axon_hooks = mod
        mod = sys.modules["antenv.axon_hooks"]
        if mod._hook is None:
            from trn_agent_boot.trn_boot import _ntff_profile_via_ctypes
            hook = _ntff_profile_via_ctypes("/opt/axon/libaxon_pjrt.so")
            if hook is None:
                return False
            mod.set_axon_ntff_profile_hook(hook)
        return True
    except Exception as e:  # profiling is best-effort
        print(f"NTFF hook install failed: {e}", file=sys.stderr)
        return False


B, C, D = 16384, 4096, 1024
N_CORES = 8
BS = B // N_CORES            # 2048 feat rows per core
KT = D // 128                # 8 k-tiles of 128
KS = KT // 2                 # 4 DoubleRow k-super-tiles of 256
MT = BS // 128               # 16 m-tiles per core
NB = 4                       # n-blocks (passes over n)
CB = C // NB                 # 1024 n-columns per block
NT = CB // 512               # 2 n-tiles of 512 per block
M_SUPER = 2                  # m-tiles per featT tile (256 cols)
SM = MT // M_SUPER           # 8 featT super-tiles

F32 = mybir.dt.float32
F16 = mybir.dt.float16
F32R = mybir.dt.float32r
FP8 = mybir.dt.float8e4
DR = mybir.MatmulPerfMode.DoubleRow

LAST = {"exec_time_ns": None, "mean_exec_time_ns": None}


def _build():
    nc = bacc.Bacc("TRN2", target_bir_lowering=False, debug=False,
                   num_devices=N_CORES)
    # DRAM layouts match the SBUF tile layouts exactly (host pre-arranged)
    d_featT = nc.dram_tensor("featT", [128, SM * KT * 256], FP8,
                             kind="ExternalInput").ap()
    d_centersT = nc.dram_tensor("centersT", [128, NB * KT * CB], FP8,
                                kind="ExternalInput").ap()
    d_c2b = nc.dram_tensor("c2b", [128, C], F16, kind="ExternalInput").ap()
    d_x2 = nc.dram_tensor("x2", [128, MT], F32, kind="ExternalInput").ap()
    d_dist = nc.dram_tensor("dist", [BS, C], F16, kind="ExternalOutput").ap()

    featT_src = d_featT.rearrange("p (sm kt m) -> p sm kt m", sm=SM, kt=KT)
    ct_src = d_centersT.rearrange("p (nb kt n) -> p nb kt n", nb=NB, kt=KT)

    with tile.TileContext(nc) as tc:
        with tc.tile_pool(name="cpool", bufs=1) as cpool, \
             tc.tile_pool(name="opool", bufs=8) as opool, \
             tc.tile_pool(name="psp", bufs=3, space="PSUM") as psp:
            # fully SBUF-resident fp8 operands
            fts = [cpool.tile([128, KT, 256], FP8, name=f"ft{s}")
                   for s in range(SM)]
            cts = [cpool.tile([128, KT, CB], FP8, name=f"ct{b}")
                   for b in range(NB)]
            x2all = cpool.tile([128, MT], F32, name="x2all")
            c2b = cpool.tile([128, C], F16, name="c2b")

            # head: first matmul needs ft0 kt0-1 + ct0 kt0-1 only; pace the
            # rest of block 0's k-tiles, then everything else. c2b loads per
            # n-block so the 1 MB doesn't sit ahead of later ft tiles.
            nc.sync.dma_start(fts[0][:, 0:2, :], featT_src[:, 0, 0:2, :])
            nc.sync.dma_start(cts[0][:, 0:2, :], ct_src[:, 0, 0:2, :])
            nc.sync.dma_start(fts[0][:, 2:KT, :], featT_src[:, 0, 2:KT, :])
            nc.sync.dma_start(cts[0][:, 2:4, :], ct_src[:, 0, 2:4, :])
            nc.sync.dma_start(cts[0][:, 4:8, :], ct_src[:, 0, 4:8, :])
            nc.sync.dma_start(fts[1][:], featT_src[:, 1, :, :])
            nc.sync.dma_start(x2all[:], d_x2)
            nc.sync.dma_start(fts[2][:], featT_src[:, 2, :, :])
            nc.sync.dma_start(c2b[:, 0:CB], d_c2b[:, 0:CB])
            for s in range(3, SM):
                nc.sync.dma_start(fts[s][:], featT_src[:, s, :, :])
            for b in range(1, NB):
                nc.sync.dma_start(cts[b][:], ct_src[:, b, :, :])
            nc.sync.dma_start(c2b[:, CB:C], d_c2b[:, CB:C])

            # HAM warm-up: dummy matmuls on a memset tile while the head
            # DMAs are in flight, so real matmuls run at 2.4 GHz. 256-col
            # dummies keep the quantum small so real MMs start promptly.
            wsrc = cpool.tile([128, 256], F32, name="wsrc")
            nc.vector.memset(wsrc[:], 0.5)
            wsrc_r = cpool.tile([128, 256], F32R, name="wsrc_r")
            nc.vector.tensor_copy(wsrc_r[:], wsrc[:])
            pd = psp.tile([128, 512], F32, name="pd", bufs=1)
            for w in range(12):
                nc.tensor.matmul(pd[:, 0:256], wsrc_r[:, 0:128], wsrc_r[:],
                                 start=True, stop=True)

            for pb in range(NB):
                ct = cts[pb]
                for sm in range(SM):
                    for mi in range(M_SUPER):
                        mt = sm * M_SUPER + mi
                        pss = [psp.tile([128, 512], F32, name=f"ps{n}")
                               for n in range(NT)]
                        for kk in range(KS):
                            lhs = fts[sm][:, 2 * kk:2 * kk + 2,
                                          bass.ts(mi, 128)]
                            for n in range(NT):
                                nc.tensor.matmul(
                                    pss[n][:], lhs,
                                    ct[:, 2 * kk:2 * kk + 2, bass.ts(n, 512)],
                                    start=(kk == 0), stop=(kk == KS - 1),
                                    perf_mode=DR)
                            if pb == 0 and mt == 0 and kk < KS - 1:
                                # m-tile 0's k-loop is paced by ct DMA
                                # arrivals; fill gaps with dummy matmuls so
                                # HAM never re-throttles
                                for w in range(2):
                                    nc.tensor.matmul(pd[:, 0:256],
                                                     wsrc_r[:, 0:128],
                                                     wsrc_r[:],
                                                     start=True, stop=True)
                        osb = opool.tile([128, CB], F16, name="osb")
                        last = (pb == NB - 1 and mt == MT - 1)
                        for n in range(NT):
                            gn = pb * CB + n * 512   # global n offset
                            nc.scalar.activation(
                                osb[:, bass.ts(n, 512)], pss[n][:],
                                mybir.ActivationFunctionType.Identity,
                                bias=x2all[:, mt:mt + 1], scale=-2.0)
                            nc.vector.tensor_add(osb[:, bass.ts(n, 512)],
                                                 osb[:, bass.ts(n, 512)],
                                                 c2b[:, gn:gn + 512])
                            if last:
                                # final m-tile: half-DMAs right after each
                                # 512-slice epilogue to shorten the tail
                                nc.sync.dma_start(
                                    d_dist[bass.ts(mt, 128), gn:gn + 512],
                                    osb[:, bass.ts(n, 512)])
                        if not last:
                            nc.sync.dma_start(
                                d_dist[bass.ts(mt, 128), bass.ts(pb, CB)],
                                osb[:])

            # sink read so the warm-up/dummy matmuls aren't dead-code
            wsink = cpool.tile([128, 1], F32, name="wsink")
            nc.scalar.copy(wsink[:], pd[:, 0:1])

    nc.compile()
    return nc


def kernel(feat: np.ndarray, centers: np.ndarray, *, trace: bool = False) -> np.ndarray:
    feat = np.ascontiguousarray(np.asarray(feat, dtype=np.float32))
    centers = np.ascontiguousarray(np.asarray(centers, dtype=np.float32))
    assert feat.shape == (B, D) and centers.shape == (C, D)

    FP8NP = ml_dtypes.float8_e4m3          # TRN e4m3 variant (max +-240)
    feat8 = feat.astype(FP8NP)             # RNE; |x| <= ~6 so no overflow
    centers8 = centers.astype(FP8NP)

    # exact row norms (the fp8 rounding only affects the cross term)
    c2 = (centers.astype(np.float64) ** 2).sum(axis=1).astype(np.float32)
    c2b = np.ascontiguousarray(
        np.broadcast_to(c2[None, :], (128, C))).astype(np.float16)
    x2 = (feat.astype(np.float64) ** 2).sum(axis=1).astype(np.float32)

    # centersT arranged [p][nb][kt][n'] so block DMAs are contiguous
    ctT = np.ascontiguousarray(centers8.T)                   # [D, C]
    ct_arr = np.ascontiguousarray(
        ctT.reshape(KT, 128, NB, CB).transpose(1, 2, 0, 3)).reshape(128, -1)

    in_maps = []
    for i in range(N_CORES):
        sl = slice(i * BS, (i + 1) * BS)
        ftT = np.ascontiguousarray(feat8[sl].T)              # [D, BS]
        ft_arr = np.ascontiguousarray(
            ftT.reshape(KT, 128, SM, 256).transpose(1, 2, 0, 3)
        ).reshape(128, -1)
        in_maps.append({
            "featT": ft_arr,
            "centersT": ct_arr,
            "c2b": c2b,
            # x2 shard laid out [128, MT]: column mt holds rows of m-tile mt
            "x2": np.ascontiguousarray(x2[sl].reshape(MT, 128).T),
        })

    if trace:
        trace = _install_ntff_hook()

    nc = _build()
    res = None
    for attempt in range(3):
        try:
            res = run_bass_kernel_spmd(nc, in_maps,
                                       core_ids=list(range(N_CORES)),
                                       trace=trace)
            break
        except Exception as e:
            # transient NRT/axon device faults recover on retry
            if attempt == 2:
                raise
            print(f"kernel run attempt {attempt} failed ({e}); retrying",
                  file=sys.stderr)
    LAST["exec_time_ns"] = res.exec_time_ns
    LAST["mean_exec_time_ns"] = res.mean_exec_time_ns

    out = np.empty((B, C), dtype=np.float32)
    for i in range(N_CORES):
        out[i * BS:(i + 1) * BS] = res.results[i]["dist"].astype(np.float32)
    return out


if __name__ == "__main__":
    rng = np.random.default_rng(0)
    f = rng.standard_normal((B, D), dtype=np.float32)
    c = rng.standard_normal((C, D), dtype=np.float32)
    d = kernel(f, c, trace=True)
    print("exec_time_ns:", LAST["exec_time_ns"])
